# revision 10
# baseline (speedup 1.0000x reference)
"""Performer block (FAVOR+ causal) Trainium2 kernel, 8-way SPMD.

Sharding: stage 1+2 head-parallel (2 heads/core, all tokens), ReduceScatter
on attention output-projection partials, stage 4 token-parallel (512 tok/core).
"""
import sys

for _p in ("/root/.axon_site", "/root/.axon_site/_ro/trn_rl_repo",
           "/root/.axon_site/_ro/pypackages", "/opt/trn_rl_repo"):
    if _p not in sys.path:
        sys.path.append(_p)

import numpy as np
import concourse.bacc as bacc
import concourse.tile as tile
from concourse import mybir
from concourse.bass_utils import run_bass_kernel_spmd

F32 = mybir.dt.float32
N_CORES = 8
B, S, D, H, MLP = 2, 2048, 1024, 16, 4096
DH = D // H                 # 64
T = B * S                   # 4096 tokens
HPC = H // N_CORES          # 2 heads per core
M2 = HPC * DH               # 128 = stacked head dim per core
TSH = T // N_CORES          # 512 tokens per core shard
P = 128
ND = D // P                 # 8 d-chunks
NMC = MLP // P              # 32 mlp chunks
NCH = S // P                # 16 chunks per batch
EPS_LN = 1e-6
KEPS = 1e-3

_CACHE = {}
LAST_RESULT = None


def _layernorm_xhat(nc, pool, eps_sb, x_t, out):
    """out = (x - mean) * rsqrt(var + eps), stats over free dim (D=1024)."""
    stats = pool.tile([P, 2, 6], F32, tag="bnstats")
    xg = x_t.rearrange("p (g d) -> p g d", g=2)
    for g in range(2):
        nc.vector.bn_stats(out=stats[:, g, :], in_=xg[:, g, :])
    mv = pool.tile([P, 2], F32, tag="bnaggr")
    nc.vector.bn_aggr(out=mv, in_=stats)
    nc.scalar.activation(
        out=mv[:, 1:2], in_=mv[:, 1:2],
        func=mybir.ActivationFunctionType.Sqrt,
        bias=eps_sb, scale=1.0)
    nc.vector.reciprocal(out=mv[:, 1:2], in_=mv[:, 1:2])
    nc.vector.tensor_scalar(
        out=out, in0=x_t, scalar1=mv[:, 0:1], scalar2=mv[:, 1:2],
        op0=mybir.AluOpType.subtract, op1=mybir.AluOpType.mult)


def _transpose_1024(nc, tp_ps, ident, src, dst_blk, col0):
    """PE-transpose src [128,1024] into dst_blk[:, d, col0:col0+128] (d=0..7)."""
    tp = tp_ps.tile([P, 4, P], F32)
    for dk in range(ND):
        nc.tensor.transpose(tp[:, dk % 4, :], src[:, dk * P:(dk + 1) * P],
                            ident)
        if dk % 4 == 3:
            nc.vector.tensor_copy(
                out=dst_blk[:, dk - 3:dk + 1, col0:col0 + P], in_=tp)
            if dk == 3:
                tp = tp_ps.tile([P, 4, P], F32)


def _build(flags):
    have_ln1b, have_b2 = flags
    nc = bacc.Bacc("TRN2", target_bir_lowering=False, debug=False,
                   num_devices=N_CORES)

    def inp(name, shape):
        return nc.dram_tensor(name, list(shape), F32, kind="ExternalInput").ap()

    x_dram = inp("x", (T, D))
    xsh_dram = inp("xsh", (TSH, D))          # this core's token shard of inputs
    wq_dram = inp("wq", (P, ND, P))          # [p, d, m] pre-tiled
    wk_dram = inp("wk", (P, ND, P))
    wv_dram = inp("wv", (P, ND, P))
    wo_dram = inp("wo", (M2, D))
    w1_dram = inp("w1", (NMC, P, ND, P))     # [mc, p, d, j] pre-tiled
    b1_dram = inp("b1", (NMC, P))            # gelu bias per mlp unit
    w2_dram = inp("w2", (MLP, D))
    ident_dram = inp("ident", (P, P))
    lmask_dram = inp("lmask", (P, P))        # [j, i] = 1.0 if j <= i
    if have_ln1b:
        qbc_dram = inp("qbc", (M2, 1))
        kbc_dram = inp("kbc", (M2, 1))
        kbr_dram = inp("kbr", (1, M2))
        vbr_dram = inp("vbr", (1, M2))
    if have_b2:
        b2_dram = inp("b2", (1, D))
    out_dram = nc.dram_tensor("out", [TSH, D], F32, kind="ExternalOutput").ap()

    with tile.TileContext(nc) as tc:
        with (
            tc.tile_pool(name="dram", bufs=1, space="DRAM") as dram,
            tc.tile_pool(name="const", bufs=1) as const,
        ):
            partial = dram.tile([T, D], F32)
            xshard = dram.tile([TSH, D], F32)

            ident = const.tile([P, P], F32)
            nc.sync.dma_start(out=ident, in_=ident_dram)
            lmask = const.tile([P, P], F32)
            nc.sync.dma_start(out=lmask, in_=lmask_dram)
            wq_sb = const.tile([P, ND, P], F32)
            nc.sync.dma_start(out=wq_sb, in_=wq_dram)
            wk_sb = const.tile([P, ND, P], F32)
            nc.sync.dma_start(out=wk_sb, in_=wk_dram)
            wv_sb = const.tile([P, ND, P], F32)
            nc.sync.dma_start(out=wv_sb, in_=wv_dram)
            wo_sb = const.tile([M2, D], F32)
            nc.sync.dma_start(out=wo_sb, in_=wo_dram)
            eps_sb = const.tile([P, 1], F32)
            nc.vector.memset(eps_sb, EPS_LN)
            if have_ln1b:
                qbc_sb = const.tile([M2, 1], F32)
                nc.sync.dma_start(out=qbc_sb, in_=qbc_dram)
                kbc_sb = const.tile([M2, 1], F32)
                nc.sync.dma_start(out=kbc_sb, in_=kbc_dram)
                kbr_sb = const.tile([1, M2], F32)
                nc.sync.dma_start(out=kbr_sb, in_=kbr_dram)
                vbr_sb = const.tile([1, M2], F32)
                nc.sync.dma_start(out=vbr_sb, in_=vbr_dram)
            if have_b2:
                b2_sb = const.tile([P, D], F32)
                nc.sync.dma_start(out=b2_sb, in_=b2_dram.to_broadcast([P, D]))

            # ---------------- stages 1-3 (head-parallel) ------------------
            with (
                tc.tile_pool(name="xin", bufs=3) as xin_pool,
                tc.tile_pool(name="stats", bufs=4) as stats_pool,
                tc.tile_pool(name="xt", bufs=2) as xt_pool,
                tc.tile_pool(name="tp_ps", bufs=1, space="PSUM") as tp_ps,
                tc.tile_pool(name="mm_ps", bufs=2, space="PSUM") as mm_ps,
                tc.tile_pool(name="qk", bufs=1) as qk_pool,
                tc.tile_pool(name="a_ps", bufs=1, space="PSUM") as a_ps,
                tc.tile_pool(name="ot_ps", bufs=2, space="PSUM") as ot_ps,
                tc.tile_pool(name="st_ps", bufs=2, space="PSUM") as st_ps,
                tc.tile_pool(name="small", bufs=4) as small,
                tc.tile_pool(name="po", bufs=3) as po_pool,
            ):
                qt_phi = qk_pool.tile([P, T], F32)    # [2 heads x 64, tok]
                kt_phi = qk_pool.tile([P, T], F32)
                knat = qk_pool.tile([P, T // P, M2], F32)   # [tok, tile, m]
                vones = qk_pool.tile([P, T // P, HPC, DH + 1], F32)
                onat = qk_pool.tile([P, T // P, M2], F32)  # attn [tok, h*dh]
                otn = qk_pool.tile([P, T], F32)       # normalized attn^T

                for blk in range(T // 512):
                    xt_blk = xt_pool.tile([P, ND, 512], F32)
                    for lt in range(4):
                        t_idx = blk * 4 + lt
                        r0 = t_idx * P
                        x_t = xin_pool.tile([P, D], F32)
                        nc.sync.dma_start(out=x_t, in_=x_dram[r0:r0 + P, :])
                        xh = xin_pool.tile([P, D], F32, tag="xhat")
                        _layernorm_xhat(nc, stats_pool, eps_sb, x_t, xh)
                        _transpose_1024(nc, tp_ps, ident, xh, xt_blk, lt * P)

                    c0 = blk * 512
                    # Qt / Kt -> phi, [m=128, 512]
                    for w_sb, dst, bc in (
                            (wq_sb, qt_phi, "q"), (wk_sb, kt_phi, "k")):
                        ps = mm_ps.tile([P, 512], F32, tag="mm")
                        for dk in range(ND):
                            nc.tensor.matmul(ps, w_sb[:, dk, :],
                                             xt_blk[:, dk, :],
                                             start=(dk == 0),
                                             stop=(dk == ND - 1))
                        if have_ln1b:
                            bias = qbc_sb if bc == "q" else kbc_sb
                            nc.scalar.activation(
                                out=dst[:, c0:c0 + 512], in_=ps,
                                func=mybir.ActivationFunctionType.Relu,
                                bias=bias, scale=1.0)
                            nc.vector.tensor_scalar_add(
                                dst[:, c0:c0 + 512], dst[:, c0:c0 + 512],
                                KEPS)
                        else:
                            nc.vector.tensor_scalar(
                                out=dst[:, c0:c0 + 512], in0=ps,
                                scalar1=0.0, scalar2=KEPS,
                                op0=mybir.AluOpType.max,
                                op1=mybir.AluOpType.add)
                    # K_nat / V_nat per 128-token tile
                    for lt in range(4):
                        t_idx = blk * 4 + lt
                        kps = mm_ps.tile([P, M2], F32, tag="mm")
                        for dk in range(ND):
                            nc.tensor.matmul(
                                kps, xt_blk[:, dk, lt * P:(lt + 1) * P],
                                wk_sb[:, dk, :],
                                start=(dk == 0), stop=(dk == ND - 1))
                        if have_ln1b:
                            nc.vector.tensor_tensor(
                                out=knat[:, t_idx, :], in0=kps,
                                in1=kbr_sb.to_broadcast([P, M2]),
                                op=mybir.AluOpType.add)
                            nc.vector.tensor_scalar(
                                out=knat[:, t_idx, :], in0=knat[:, t_idx, :],
                                scalar1=0.0, scalar2=KEPS,
                                op0=mybir.AluOpType.max,
                                op1=mybir.AluOpType.add)
                        else:
                            nc.vector.tensor_scalar(
                                out=knat[:, t_idx, :], in0=kps,
                                scalar1=0.0, scalar2=KEPS,
                                op0=mybir.AluOpType.max,
                                op1=mybir.AluOpType.add)
                        vps = mm_ps.tile([P, M2], F32, tag="mm")
                        for dk in range(ND):
                            nc.tensor.matmul(
                                vps, xt_blk[:, dk, lt * P:(lt + 1) * P],
                                wv_sb[:, dk, :],
                                start=(dk == 0), stop=(dk == ND - 1))
                        for h in range(HPC):
                            if have_ln1b:
                                nc.vector.tensor_tensor(
                                    out=vones[:, t_idx, h, 0:DH],
                                    in0=vps[:, h * DH:(h + 1) * DH],
                                    in1=vbr_sb[:, h * DH:(h + 1) * DH]
                                        .to_broadcast([P, DH]),
                                    op=mybir.AluOpType.add)
                            else:
                                nc.vector.tensor_copy(
                                    out=vones[:, t_idx, h, 0:DH],
                                    in_=vps[:, h * DH:(h + 1) * DH])
                            nc.vector.memset(
                                vones[:, t_idx, h, DH:DH + 1], 1.0)

                # ---------------- attention (chunked causal scan) ---------
                # per chunk, head: O_nat [i, dh+1] = (A'∘L)ᵀ-style natural
                # orientation: intra = lhsT(am[j,i]).T @ vones[j,:] and
                # inter = lhsT(qt[m,i]).T @ state[m,:]; col 64 = denominator.
                for b in range(B):
                    # both heads' scan state in one 128-partition tile:
                    # rows 64h..64h+63 belong to head h (keeps matmul operand
                    # base partitions aligned with qt_phi/kt_phi head slices)
                    stp = st_ps.tile([P, DH + 1], F32, tag="stp",
                                     name=f"stp{b}")
                    state_sb = None
                    for ch in range(NCH):
                        g0 = b * S + ch * P
                        t_idx = g0 // P
                        cs = slice(g0, g0 + P)
                        prev_state = state_sb
                        if ch < NCH - 1:
                            state_sb = small.tile([P, DH + 1], F32,
                                                  tag="state",
                                                  name=f"state{b}_{ch}")
                        for h in range(HPC):
                            hr = slice(h * DH, (h + 1) * DH)
                            aps = a_ps.tile([P, P], F32)
                            nc.tensor.matmul(aps, kt_phi[hr, cs],
                                             qt_phi[hr, cs],
                                             start=True, stop=True)
                            am = small.tile([P, P], F32, tag="am")
                            nc.vector.tensor_tensor(
                                out=am, in0=aps, in1=lmask,
                                op=mybir.AluOpType.mult)
                            ops = ot_ps.tile([P, DH + 1], F32)
                            nc.tensor.matmul(ops, am, vones[:, t_idx, h, :],
                                             start=True, stop=(ch == 0))
                            if ch > 0:
                                nc.tensor.matmul(ops, qt_phi[hr, cs],
                                                 prev_state[hr, :],
                                                 start=False, stop=True,
                                                 skip_group_check=True)
                            if ch < NCH - 1:
                                nc.tensor.matmul(
                                    stp[hr, :], knat[:, t_idx, hr],
                                    vones[:, t_idx, h, :],
                                    start=(ch == 0), stop=(ch == NCH - 2),
                                    skip_group_check=True)
                                nc.vector.tensor_copy(out=state_sb[hr, :],
                                                      in_=stp[hr, :])
                            rec = small.tile([P, 1], F32, tag="rec")
                            nc.vector.reciprocal(out=rec,
                                                 in_=ops[:, DH:DH + 1])
                            nc.vector.tensor_scalar_mul(
                                onat[:, t_idx, hr], ops[:, 0:DH], rec)
                        # transpose both heads [tok,128] -> otn [128, tok]
                        otp = tp_ps.tile([P, P], F32, tag="tp")
                        nc.tensor.transpose(otp, onat[:, t_idx, :], ident)
                        nc.vector.tensor_copy(out=otn[:, cs], in_=otp)

                # ---------------- output projection partials --------------
                for t_idx in range(T // P):
                    r0 = t_idx * P
                    posb = po_pool.tile([P, D], F32)
                    for half in range(2):
                        pops = mm_ps.tile([P, 512], F32, tag="mm")
                        nc.tensor.matmul(
                            pops, otn[:, r0:r0 + P],
                            wo_sb[:, half * 512:(half + 1) * 512],
                            start=True, stop=True)
                        nc.vector.tensor_copy(
                            out=posb[:, half * 512:(half + 1) * 512],
                            in_=pops)
                    nc.sync.dma_start(out=partial[r0:r0 + P, :], in_=posb)

            nc.gpsimd.collective_compute(
                "ReduceScatter",
                mybir.AluOpType.add,
                replica_groups=[list(range(N_CORES))],
                ins=[partial.opt()],
                outs=[xshard.opt()],
            )

            # ---------------- stage 4: token-local MLP --------------------
            with (
                tc.tile_pool(name="xs", bufs=4) as xs_pool,
                tc.tile_pool(name="ys", bufs=2) as ys_pool,
                tc.tile_pool(name="stats2", bufs=4) as stats2_pool,
                tc.tile_pool(name="yt", bufs=1) as yt_pool,
                tc.tile_pool(name="tp2_ps", bufs=2, space="PSUM") as tp2_ps,
                tc.tile_pool(name="h_sb", bufs=1) as h_pool,
                tc.tile_pool(name="w1t", bufs=3) as w1t_pool,
                tc.tile_pool(name="w2t", bufs=3) as w2t_pool,
                tc.tile_pool(name="b1t", bufs=3) as b1t_pool,
                tc.tile_pool(name="h_ps", bufs=2, space="PSUM") as h_ps,
                tc.tile_pool(name="y2_ps", bufs=4, space="PSUM") as y2_ps,
                tc.tile_pool(name="fin", bufs=4) as fin_pool,
            ):
                yt_blk = yt_pool.tile([P, ND, TSH], F32)
                x_tiles = []
                for lt in range(TSH // P):
                    r0 = lt * P
                    xs = xs_pool.tile([P, D], F32)
                    nc.sync.dma_start(out=xs, in_=xshard[r0:r0 + P, :])
                    xres = xs_pool.tile([P, D], F32, tag="xres")
                    nc.sync.dma_start(out=xres, in_=xsh_dram[r0:r0 + P, :])
                    nc.vector.tensor_add(xs, xs, xres)
                    x_tiles.append(xs)
                    ysb = ys_pool.tile([P, D], F32)
                    _layernorm_xhat(nc, stats2_pool, eps_sb, xs, ysb)
                    _transpose_1024(nc, tp2_ps, ident, ysb, yt_blk, r0)

                h_all = h_pool.tile([P, NMC, TSH], F32)
                for mc in range(NMC):
                    w1t = w1t_pool.tile([P, ND, P], F32)
                    nc.sync.dma_start(out=w1t, in_=w1_dram[mc])
                    b1t = b1t_pool.tile([P, 1], F32)
                    nc.sync.dma_start(
                        out=b1t, in_=b1_dram[mc].rearrange("(p o) -> p o", o=1))
                    hps = h_ps.tile([P, TSH], F32)
                    for dk in range(ND):
                        nc.tensor.matmul(hps, w1t[:, dk, :], yt_blk[:, dk, :],
                                         start=(dk == 0), stop=(dk == ND - 1))
                    nc.scalar.activation(
                        out=h_all[:, mc, :], in_=hps,
                        func=mybir.ActivationFunctionType.Gelu_apprx_tanh,
                        bias=b1t, scale=1.0)

                fins = [fin_pool.tile([P, D], F32, tag="fin", name=f"fin{_l}")
                        for _l in range(TSH // P)]
                for half in range(2):
                    hs0 = half * 512
                    y2p = [y2_ps.tile([P, 512], F32, tag="y2", name=f"y2_{half}_{_l}")
                           for _l in range(TSH // P)]
                    for mc in range(NMC):
                        w2t = w2t_pool.tile([P, 512], F32)
                        nc.sync.dma_start(
                            out=w2t,
                            in_=w2_dram[mc * P:(mc + 1) * P, hs0:hs0 + 512])
                        for lt in range(TSH // P):
                            r0 = lt * P
                            nc.tensor.matmul(
                                y2p[lt], h_all[:, mc, r0:r0 + P], w2t,
                                start=(mc == 0), stop=(mc == NMC - 1))
                    for lt in range(TSH // P):
                        nc.vector.tensor_tensor(
                            out=fins[lt][:, hs0:hs0 + 512], in0=y2p[lt],
                            in1=x_tiles[lt][:, hs0:hs0 + 512],
                            op=mybir.AluOpType.add)
                        if have_b2:
                            nc.vector.tensor_add(
                                fins[lt][:, hs0:hs0 + 512],
                                fins[lt][:, hs0:hs0 + 512],
                                b2_sb[:, hs0:hs0 + 512])
                for lt in range(TSH // P):
                    nc.sync.dma_start(out=out_dram[lt * P:(lt + 1) * P, :],
                                      in_=fins[lt])

    nc.compile()
    return nc


def kernel(**inputs):
    inp = np.asarray(inputs["inputs"], np.float32).reshape(T, D)
    ln1s = np.asarray(inputs["ln1_scale"], np.float32)
    ln1b = np.asarray(inputs["ln1_bias"], np.float32)
    wq = np.asarray(inputs["wq"], np.float32).reshape(D, H * DH)
    wk = np.asarray(inputs["wk"], np.float32).reshape(D, H * DH)
    wv = np.asarray(inputs["wv"], np.float32).reshape(D, H * DH)
    wo = np.asarray(inputs["wo"], np.float32).reshape(H * DH, D)
    ln2s = np.asarray(inputs["ln2_scale"], np.float32)
    ln2b = np.asarray(inputs["ln2_bias"], np.float32)
    w1 = np.asarray(inputs["w1"], np.float32)
    b1 = np.asarray(inputs["b1"], np.float32)
    w2 = np.asarray(inputs["w2"], np.float32)
    b2 = np.asarray(inputs["b2"], np.float32)

    wq_e = ln1s[:, None] * wq
    wk_e = ln1s[:, None] * wk
    wv_e = ln1s[:, None] * wv
    qb = ln1b @ wq
    kb = ln1b @ wk
    vb = ln1b @ wv
    w1_e = ln2s[:, None] * w1
    b1_e = ln2b @ w1 + b1

    have_ln1b = bool(np.any(qb) or np.any(kb) or np.any(vb))
    have_b2 = bool(np.any(b2))
    key = (have_ln1b, have_b2)
    if key not in _CACHE:
        _CACHE[key] = _build(key)
    nc = _CACHE[key]

    def tile_w(w):  # [D, 128] -> [p, d, j] pre-tiled
        return np.ascontiguousarray(w.reshape(ND, P, P).transpose(1, 0, 2))

    w1_tiled = np.ascontiguousarray(
        w1_e.reshape(ND, P, NMC, P).transpose(2, 1, 0, 3))
    in_maps = []
    for c in range(N_CORES):
        hs = slice(c * M2, (c + 1) * M2)
        m = {
            "x": inp,
            "xsh": np.ascontiguousarray(inp[c * TSH:(c + 1) * TSH]),
            "wq": tile_w(wq_e[:, hs]),
            "wk": tile_w(wk_e[:, hs]),
            "wv": tile_w(wv_e[:, hs]),
            "wo": np.ascontiguousarray(wo[hs, :]),
            "w1": w1_tiled,
            "b1": np.ascontiguousarray(b1_e.reshape(NMC, P)),
            "w2": w2,
            "ident": np.eye(P, dtype=np.float32),
            "lmask": np.ascontiguousarray(
                np.tril(np.ones((P, P), np.float32)).T),
        }
        if have_ln1b:
            m["qbc"] = np.ascontiguousarray(qb[hs].reshape(M2, 1))
            m["kbc"] = np.ascontiguousarray(kb[hs].reshape(M2, 1))
            m["kbr"] = np.ascontiguousarray(kb[hs].reshape(1, M2))
            m["vbr"] = np.ascontiguousarray(vb[hs].reshape(1, M2))
        if have_b2:
            m["b2"] = b2.reshape(1, D)
        in_maps.append(m)

    global LAST_RESULT
    res = run_bass_kernel_spmd(nc, in_maps, core_ids=list(range(N_CORES)))
    LAST_RESULT = res
    out = np.concatenate([res.results[c]["out"] for c in range(N_CORES)], 0)
    return out.reshape(B, S, D).astype(np.float32)


# revision 12
# speedup vs baseline: 2.5102x; 2.5102x over previous
"""Performer block (FAVOR+ causal) Trainium2 kernel, 8-way SPMD.

Sharding: stage 1+2 head-parallel (2 heads/core, all tokens), ReduceScatter
on attention output-projection partials, stage 4 token-parallel (512 tok/core).
Compute in bf16 (PSUM accumulation fp32); LN stats and normalization in fp32.
"""
import sys

for _p in ("/root/.axon_site", "/root/.axon_site/_ro/trn_rl_repo",
           "/root/.axon_site/_ro/pypackages", "/opt/trn_rl_repo"):
    if _p not in sys.path:
        sys.path.append(_p)

import ml_dtypes
import numpy as np
import concourse.bacc as bacc
import concourse.tile as tile
from concourse import mybir
from concourse.bass_utils import run_bass_kernel_spmd

F32 = mybir.dt.float32
BF16 = mybir.dt.bfloat16
NP_BF16 = ml_dtypes.bfloat16
N_CORES = 8
B, S, D, H, MLP = 2, 2048, 1024, 16, 4096
DH = D // H                 # 64
T = B * S                   # 4096 tokens
HPC = H // N_CORES          # 2 heads per core
M2 = HPC * DH               # 128 = stacked head dim per core
TSH = T // N_CORES          # 512 tokens per core shard
P = 128
ND = D // P                 # 8 d-chunks
NMC = MLP // P              # 32 mlp chunks
NCH = S // P                # 16 chunks per batch
EPS_LN = 1e-6
KEPS = 1e-3

_CACHE = {}
LAST_RESULT = None


def _layernorm_xhat(nc, pool, eps_sb, x_t, out):
    """out = (x - mean) * rsqrt(var + eps), stats over free dim (D=1024)."""
    stats = pool.tile([P, 2, 6], F32, tag="bnstats")
    xg = x_t.rearrange("p (g d) -> p g d", g=2)
    for g in range(2):
        nc.vector.bn_stats(out=stats[:, g, :], in_=xg[:, g, :])
    mv = pool.tile([P, 2], F32, tag="bnaggr")
    nc.vector.bn_aggr(out=mv, in_=stats)
    nc.scalar.activation(
        out=mv[:, 1:2], in_=mv[:, 1:2],
        func=mybir.ActivationFunctionType.Sqrt,
        bias=eps_sb, scale=1.0)
    nc.vector.reciprocal(out=mv[:, 1:2], in_=mv[:, 1:2])
    nc.vector.tensor_scalar(
        out=out, in0=x_t, scalar1=mv[:, 0:1], scalar2=mv[:, 1:2],
        op0=mybir.AluOpType.subtract, op1=mybir.AluOpType.mult)


def _transpose_1024(nc, tp_ps, ident, src, dst_blk, col0, copy_engine=None):
    """PE-transpose src [128,1024] into dst_blk[:, d, col0:col0+128] (d=0..7)."""
    tp = tp_ps.tile([P, 4, P], BF16)
    for dk in range(ND):
        nc.tensor.transpose(tp[:, dk % 4, :], src[:, dk * P:(dk + 1) * P],
                            ident)
        if dk % 4 == 3:
            dst = dst_blk[:, dk - 3:dk + 1, col0:col0 + P]
            if copy_engine == "scalar":
                nc.scalar.activation(
                    out=dst, in_=tp,
                    func=mybir.ActivationFunctionType.Copy)
            else:
                nc.vector.tensor_copy(out=dst, in_=tp)
            if dk == 3:
                tp = tp_ps.tile([P, 4, P], BF16)


def _build(flags):
    have_ln1b, have_b2 = flags
    nc = bacc.Bacc("TRN2", target_bir_lowering=False, debug=False,
                   num_devices=N_CORES)

    def inp(name, shape, dt=BF16):
        return nc.dram_tensor(name, list(shape), dt, kind="ExternalInput").ap()

    x_dram = inp("x", (T, D), F32)
    xsh_dram = inp("xsh", (TSH, D), F32)     # this core's token shard of inputs
    wq_dram = inp("wq", (P, ND, P))          # [p, d, m] pre-tiled, bf16
    wk_dram = inp("wk", (P, ND, P))
    wv_dram = inp("wv", (P, ND, P))
    wo_dram = inp("wo", (M2, D))
    w1_dram = inp("w1", (NMC, P, ND, P))     # [mc, p, d, j] pre-tiled, bf16
    b1_dram = inp("b1", (NMC, P), F32)       # gelu bias per mlp unit
    w2_dram = inp("w2", (MLP, D))
    ident_dram = inp("ident", (P, P))
    lmask_dram = inp("lmask", (P, P), F32)   # [j, i] = 1.0 if j <= i
    if have_ln1b:
        qbc_dram = inp("qbc", (M2, 1), F32)
        kbc_dram = inp("kbc", (M2, 1), F32)
        kbr_dram = inp("kbr", (1, M2), F32)
        vbr_dram = inp("vbr", (1, M2), F32)
    if have_b2:
        b2_dram = inp("b2", (1, D), F32)
    out_dram = nc.dram_tensor("out", [TSH, D], F32, kind="ExternalOutput").ap()

    with tile.TileContext(nc) as tc:
        with (
            tc.tile_pool(name="dram", bufs=1, space="DRAM") as dram,
            tc.tile_pool(name="const", bufs=1) as const,
        ):
            partial = dram.tile([T, D], BF16)
            xshard = dram.tile([TSH, D], BF16)

            ident = const.tile([P, P], BF16)
            nc.sync.dma_start(out=ident, in_=ident_dram)
            lmask = const.tile([P, P], F32)
            nc.sync.dma_start(out=lmask, in_=lmask_dram)
            wq_sb = const.tile([P, ND, P], BF16)
            nc.sync.dma_start(out=wq_sb, in_=wq_dram)
            wk_sb = const.tile([P, ND, P], BF16)
            nc.sync.dma_start(out=wk_sb, in_=wk_dram)
            wv_sb = const.tile([P, ND, P], BF16)
            nc.sync.dma_start(out=wv_sb, in_=wv_dram)
            wo_sb = const.tile([M2, D], BF16)
            nc.sync.dma_start(out=wo_sb, in_=wo_dram)
            eps_sb = const.tile([P, 1], F32)
            nc.vector.memset(eps_sb, EPS_LN)
            if have_ln1b:
                qbc_sb = const.tile([M2, 1], F32)
                nc.sync.dma_start(out=qbc_sb, in_=qbc_dram)
                kbc_sb = const.tile([M2, 1], F32)
                nc.sync.dma_start(out=kbc_sb, in_=kbc_dram)
                kbr_sb = const.tile([1, M2], F32)
                nc.sync.dma_start(out=kbr_sb, in_=kbr_dram)
                vbr_sb = const.tile([1, M2], F32)
                nc.sync.dma_start(out=vbr_sb, in_=vbr_dram)
            if have_b2:
                b2_sb = const.tile([P, D], F32)
                nc.sync.dma_start(out=b2_sb, in_=b2_dram.to_broadcast([P, D]))

            # ---------------- stages 1-3 (head-parallel) ------------------
            with (
                tc.tile_pool(name="xin", bufs=3) as xin_pool,
                tc.tile_pool(name="stats", bufs=4) as stats_pool,
                tc.tile_pool(name="xt", bufs=2) as xt_pool,
                tc.tile_pool(name="tp_ps", bufs=1, space="PSUM") as tp_ps,
                tc.tile_pool(name="mm_ps", bufs=2, space="PSUM") as mm_ps,
                tc.tile_pool(name="qk", bufs=1) as qk_pool,
                tc.tile_pool(name="a_ps", bufs=1, space="PSUM") as a_ps,
                tc.tile_pool(name="ot_ps", bufs=2, space="PSUM") as ot_ps,
                tc.tile_pool(name="st_ps", bufs=2, space="PSUM") as st_ps,
                tc.tile_pool(name="small", bufs=4) as small,
                tc.tile_pool(name="po", bufs=3) as po_pool,
            ):
                qt_phi = qk_pool.tile([P, T], BF16)    # [2 heads x 64, tok]
                kt_phi = qk_pool.tile([P, T], BF16)
                knat = qk_pool.tile([P, T // P, M2], BF16)  # [tok, tile, m]
                vones = qk_pool.tile([P, T // P, HPC, DH + 1], BF16)
                onat = qk_pool.tile([P, T // P, M2], BF16)  # attn [tok, h*dh]
                otn = qk_pool.tile([P, T], BF16)       # normalized attn^T

                for blk in range(T // 512):
                    xt_blk = xt_pool.tile([P, ND, 512], BF16)
                    for lt in range(4):
                        t_idx = blk * 4 + lt
                        r0 = t_idx * P
                        x_t = xin_pool.tile([P, D], F32)
                        nc.sync.dma_start(out=x_t, in_=x_dram[r0:r0 + P, :])
                        xh = xin_pool.tile([P, D], BF16, tag="xhat")
                        _layernorm_xhat(nc, stats_pool, eps_sb, x_t, xh)
                        _transpose_1024(nc, tp_ps, ident, xh, xt_blk, lt * P,
                                        copy_engine="scalar" if lt % 2 else None)

                    c0 = blk * 512
                    # Qt / Kt -> phi, [m=128, 512]
                    for w_sb, dst, bc in (
                            (wq_sb, qt_phi, "q"), (wk_sb, kt_phi, "k")):
                        ps = mm_ps.tile([P, 512], F32, tag="mm")
                        for dk in range(ND):
                            nc.tensor.matmul(ps, w_sb[:, dk, :],
                                             xt_blk[:, dk, :],
                                             start=(dk == 0),
                                             stop=(dk == ND - 1))
                        if have_ln1b:
                            bias = qbc_sb if bc == "q" else kbc_sb
                            nc.scalar.activation(
                                out=dst[:, c0:c0 + 512], in_=ps,
                                func=mybir.ActivationFunctionType.Relu,
                                bias=bias, scale=1.0)
                            nc.vector.tensor_scalar_add(
                                dst[:, c0:c0 + 512], dst[:, c0:c0 + 512],
                                KEPS)
                        else:
                            nc.vector.tensor_scalar(
                                out=dst[:, c0:c0 + 512], in0=ps,
                                scalar1=0.0, scalar2=KEPS,
                                op0=mybir.AluOpType.max,
                                op1=mybir.AluOpType.add)
                    # K_nat / V_nat per 128-token tile
                    for lt in range(4):
                        t_idx = blk * 4 + lt
                        kps = mm_ps.tile([P, M2], F32, tag="mm")
                        for dk in range(ND):
                            nc.tensor.matmul(
                                kps, xt_blk[:, dk, lt * P:(lt + 1) * P],
                                wk_sb[:, dk, :],
                                start=(dk == 0), stop=(dk == ND - 1))
                        if have_ln1b:
                            nc.vector.tensor_tensor(
                                out=knat[:, t_idx, :], in0=kps,
                                in1=kbr_sb.to_broadcast([P, M2]),
                                op=mybir.AluOpType.add)
                            nc.vector.tensor_scalar(
                                out=knat[:, t_idx, :], in0=knat[:, t_idx, :],
                                scalar1=0.0, scalar2=KEPS,
                                op0=mybir.AluOpType.max,
                                op1=mybir.AluOpType.add)
                        else:
                            nc.vector.tensor_scalar(
                                out=knat[:, t_idx, :], in0=kps,
                                scalar1=0.0, scalar2=KEPS,
                                op0=mybir.AluOpType.max,
                                op1=mybir.AluOpType.add)
                        vps = mm_ps.tile([P, M2], F32, tag="mm")
                        for dk in range(ND):
                            nc.tensor.matmul(
                                vps, xt_blk[:, dk, lt * P:(lt + 1) * P],
                                wv_sb[:, dk, :],
                                start=(dk == 0), stop=(dk == ND - 1))
                        for h in range(HPC):
                            if have_ln1b:
                                nc.vector.tensor_tensor(
                                    out=vones[:, t_idx, h, 0:DH],
                                    in0=vps[:, h * DH:(h + 1) * DH],
                                    in1=vbr_sb[:, h * DH:(h + 1) * DH]
                                        .to_broadcast([P, DH]),
                                    op=mybir.AluOpType.add)
                            else:
                                nc.vector.tensor_copy(
                                    out=vones[:, t_idx, h, 0:DH],
                                    in_=vps[:, h * DH:(h + 1) * DH])
                            nc.vector.memset(
                                vones[:, t_idx, h, DH:DH + 1], 1.0)

                # ---------------- attention (chunked causal scan) ---------
                for b in range(B):
                    # both heads' scan state in one 128-partition tile:
                    # rows 64h..64h+63 belong to head h (keeps matmul operand
                    # base partitions aligned with qt_phi/kt_phi head slices)
                    stp = st_ps.tile([P, DH + 1], F32, tag="stp",
                                     name=f"stp{b}")
                    state_sb = None
                    for ch in range(NCH):
                        g0 = b * S + ch * P
                        t_idx = g0 // P
                        cs = slice(g0, g0 + P)
                        prev_state = state_sb
                        if ch < NCH - 1:
                            state_sb = small.tile([P, DH + 1], BF16,
                                                  tag="state",
                                                  name=f"state{b}_{ch}")
                        for h in range(HPC):
                            hr = slice(h * DH, (h + 1) * DH)
                            aps = a_ps.tile([P, P], F32)
                            nc.tensor.matmul(aps, kt_phi[hr, cs],
                                             qt_phi[hr, cs],
                                             start=True, stop=True)
                            am = small.tile([P, P], BF16, tag="am")
                            nc.vector.tensor_tensor(
                                out=am, in0=aps, in1=lmask,
                                op=mybir.AluOpType.mult)
                            ops = ot_ps.tile([P, DH + 1], F32)
                            nc.tensor.matmul(ops, am, vones[:, t_idx, h, :],
                                             start=True, stop=(ch == 0))
                            if ch > 0:
                                nc.tensor.matmul(ops, qt_phi[hr, cs],
                                                 prev_state[hr, :],
                                                 start=False, stop=True,
                                                 skip_group_check=True)
                            if ch < NCH - 1:
                                nc.tensor.matmul(
                                    stp[hr, :], knat[:, t_idx, hr],
                                    vones[:, t_idx, h, :],
                                    start=(ch == 0), stop=(ch == NCH - 2),
                                    skip_group_check=True)
                                nc.vector.tensor_copy(out=state_sb[hr, :],
                                                      in_=stp[hr, :])
                            rec = small.tile([P, 1], F32, tag="rec")
                            nc.vector.reciprocal(out=rec,
                                                 in_=ops[:, DH:DH + 1])
                            nc.vector.tensor_scalar_mul(
                                onat[:, t_idx, hr], ops[:, 0:DH], rec)
                        # transpose both heads [tok,128] -> otn [128, tok]
                        otp = tp_ps.tile([P, P], BF16, tag="tp")
                        nc.tensor.transpose(otp, onat[:, t_idx, :], ident)
                        nc.vector.tensor_copy(out=otn[:, cs], in_=otp)

                # ---------------- output projection partials --------------
                for t_idx in range(T // P):
                    r0 = t_idx * P
                    posb = po_pool.tile([P, D], BF16)
                    for half in range(2):
                        pops = mm_ps.tile([P, 512], F32, tag="mm")
                        nc.tensor.matmul(
                            pops, otn[:, r0:r0 + P],
                            wo_sb[:, half * 512:(half + 1) * 512],
                            start=True, stop=True)
                        nc.scalar.activation(
                            out=posb[:, half * 512:(half + 1) * 512],
                            in_=pops,
                            func=mybir.ActivationFunctionType.Copy)
                    nc.sync.dma_start(out=partial[r0:r0 + P, :], in_=posb)

            nc.gpsimd.collective_compute(
                "ReduceScatter",
                mybir.AluOpType.add,
                replica_groups=[list(range(N_CORES))],
                ins=[partial.opt()],
                outs=[xshard.opt()],
            )

            # ---------------- stage 4: token-local MLP --------------------
            with (
                tc.tile_pool(name="xs", bufs=4) as xs_pool,
                tc.tile_pool(name="ys", bufs=2) as ys_pool,
                tc.tile_pool(name="stats2", bufs=4) as stats2_pool,
                tc.tile_pool(name="yt", bufs=1) as yt_pool,
                tc.tile_pool(name="tp2_ps", bufs=2, space="PSUM") as tp2_ps,
                tc.tile_pool(name="h_sb", bufs=1) as h_pool,
                tc.tile_pool(name="w1t", bufs=3) as w1t_pool,
                tc.tile_pool(name="w2t", bufs=3) as w2t_pool,
                tc.tile_pool(name="b1t", bufs=3) as b1t_pool,
                tc.tile_pool(name="h_ps", bufs=2, space="PSUM") as h_ps,
                tc.tile_pool(name="y2_ps", bufs=4, space="PSUM") as y2_ps,
                tc.tile_pool(name="fin", bufs=4) as fin_pool,
            ):
                yt_blk = yt_pool.tile([P, ND, TSH], BF16)
                x_tiles = []
                for lt in range(TSH // P):
                    r0 = lt * P
                    xsb = xs_pool.tile([P, D], BF16, tag="xsb")
                    nc.sync.dma_start(out=xsb, in_=xshard[r0:r0 + P, :])
                    xs = xs_pool.tile([P, D], F32)
                    nc.sync.dma_start(out=xs, in_=xsh_dram[r0:r0 + P, :])
                    nc.vector.tensor_add(xs, xs, xsb)
                    x_tiles.append(xs)
                    ysb = ys_pool.tile([P, D], BF16)
                    _layernorm_xhat(nc, stats2_pool, eps_sb, xs, ysb)
                    _transpose_1024(nc, tp2_ps, ident, ysb, yt_blk, r0,
                                    copy_engine="scalar" if lt % 2 else None)

                h_all = h_pool.tile([P, NMC, TSH], BF16)
                for mc in range(NMC):
                    w1t = w1t_pool.tile([P, ND, P], BF16)
                    nc.sync.dma_start(out=w1t, in_=w1_dram[mc])
                    b1t = b1t_pool.tile([P, 1], F32)
                    nc.sync.dma_start(
                        out=b1t, in_=b1_dram[mc].rearrange("(p o) -> p o", o=1))
                    hps = h_ps.tile([P, TSH], F32)
                    for dk in range(ND):
                        nc.tensor.matmul(hps, w1t[:, dk, :], yt_blk[:, dk, :],
                                         start=(dk == 0), stop=(dk == ND - 1))
                    nc.scalar.activation(
                        out=h_all[:, mc, :], in_=hps,
                        func=mybir.ActivationFunctionType.Gelu_apprx_tanh,
                        bias=b1t, scale=1.0)

                fins = [fin_pool.tile([P, D], F32, tag="fin", name=f"fin{_l}")
                        for _l in range(TSH // P)]
                for half in range(2):
                    hs0 = half * 512
                    y2p = [y2_ps.tile([P, 512], F32, tag="y2",
                                      name=f"y2_{half}_{_l}")
                           for _l in range(TSH // P)]
                    for mc in range(NMC):
                        w2t = w2t_pool.tile([P, 512], BF16)
                        nc.sync.dma_start(
                            out=w2t,
                            in_=w2_dram[mc * P:(mc + 1) * P, hs0:hs0 + 512])
                        for lt in range(TSH // P):
                            r0 = lt * P
                            nc.tensor.matmul(
                                y2p[lt], h_all[:, mc, r0:r0 + P], w2t,
                                start=(mc == 0), stop=(mc == NMC - 1))
                    for lt in range(TSH // P):
                        nc.vector.tensor_tensor(
                            out=fins[lt][:, hs0:hs0 + 512], in0=y2p[lt],
                            in1=x_tiles[lt][:, hs0:hs0 + 512],
                            op=mybir.AluOpType.add)
                        if have_b2:
                            nc.vector.tensor_add(
                                fins[lt][:, hs0:hs0 + 512],
                                fins[lt][:, hs0:hs0 + 512],
                                b2_sb[:, hs0:hs0 + 512])
                for lt in range(TSH // P):
                    nc.sync.dma_start(out=out_dram[lt * P:(lt + 1) * P, :],
                                      in_=fins[lt])

    nc.compile()
    return nc


def kernel(**inputs):
    inp = np.asarray(inputs["inputs"], np.float32).reshape(T, D)
    ln1s = np.asarray(inputs["ln1_scale"], np.float32)
    ln1b = np.asarray(inputs["ln1_bias"], np.float32)
    wq = np.asarray(inputs["wq"], np.float32).reshape(D, H * DH)
    wk = np.asarray(inputs["wk"], np.float32).reshape(D, H * DH)
    wv = np.asarray(inputs["wv"], np.float32).reshape(D, H * DH)
    wo = np.asarray(inputs["wo"], np.float32).reshape(H * DH, D)
    ln2s = np.asarray(inputs["ln2_scale"], np.float32)
    ln2b = np.asarray(inputs["ln2_bias"], np.float32)
    w1 = np.asarray(inputs["w1"], np.float32)
    b1 = np.asarray(inputs["b1"], np.float32)
    w2 = np.asarray(inputs["w2"], np.float32)
    b2 = np.asarray(inputs["b2"], np.float32)

    wq_e = ln1s[:, None] * wq
    wk_e = ln1s[:, None] * wk
    wv_e = ln1s[:, None] * wv
    qb = ln1b @ wq
    kb = ln1b @ wk
    vb = ln1b @ wv
    w1_e = ln2s[:, None] * w1
    b1_e = ln2b @ w1 + b1

    have_ln1b = bool(np.any(qb) or np.any(kb) or np.any(vb))
    have_b2 = bool(np.any(b2))
    key = (have_ln1b, have_b2)
    if key not in _CACHE:
        _CACHE[key] = _build(key)
    nc = _CACHE[key]

    def bf(a):
        return np.ascontiguousarray(a.astype(NP_BF16))

    def tile_w(w):  # [D, 128] -> [p, d, j] pre-tiled
        return bf(w.reshape(ND, P, P).transpose(1, 0, 2))

    w1_tiled = bf(w1_e.reshape(ND, P, NMC, P).transpose(2, 1, 0, 3))
    w2_bf = bf(w2)
    in_maps = []
    for c in range(N_CORES):
        hs = slice(c * M2, (c + 1) * M2)
        m = {
            "x": inp,
            "xsh": np.ascontiguousarray(inp[c * TSH:(c + 1) * TSH]),
            "wq": tile_w(wq_e[:, hs]),
            "wk": tile_w(wk_e[:, hs]),
            "wv": tile_w(wv_e[:, hs]),
            "wo": bf(wo[hs, :]),
            "w1": w1_tiled,
            "b1": np.ascontiguousarray(b1_e.reshape(NMC, P)),
            "w2": w2_bf,
            "ident": np.eye(P, dtype=NP_BF16),
            "lmask": np.ascontiguousarray(
                np.tril(np.ones((P, P), np.float32)).T),
        }
        if have_ln1b:
            m["qbc"] = np.ascontiguousarray(qb[hs].reshape(M2, 1))
            m["kbc"] = np.ascontiguousarray(kb[hs].reshape(M2, 1))
            m["kbr"] = np.ascontiguousarray(kb[hs].reshape(1, M2))
            m["vbr"] = np.ascontiguousarray(vb[hs].reshape(1, M2))
        if have_b2:
            m["b2"] = b2.reshape(1, D)
        in_maps.append(m)

    global LAST_RESULT
    res = run_bass_kernel_spmd(nc, in_maps, core_ids=list(range(N_CORES)))
    LAST_RESULT = res
    out = np.concatenate([res.results[c]["out"] for c in range(N_CORES)], 0)
    return out.reshape(B, S, D).astype(np.float32)


# revision 14
# speedup vs baseline: 2.5880x; 1.0310x over previous
"""Performer block (FAVOR+ causal) Trainium2 kernel, 8-way SPMD.

Sharding: stage 1+2 head-parallel (2 heads/core, all tokens), ReduceScatter
on attention output-projection partials, stage 4 token-parallel (512 tok/core).
Compute in bf16 (PSUM accumulation fp32); LN stats and normalization in fp32.
"""
import sys

for _p in ("/root/.axon_site", "/root/.axon_site/_ro/trn_rl_repo",
           "/root/.axon_site/_ro/pypackages", "/opt/trn_rl_repo"):
    if _p not in sys.path:
        sys.path.append(_p)

import ml_dtypes
import numpy as np
import concourse.bacc as bacc
import concourse.tile as tile
from concourse import mybir
from concourse.bass_utils import run_bass_kernel_spmd

F32 = mybir.dt.float32
BF16 = mybir.dt.bfloat16
NP_BF16 = ml_dtypes.bfloat16
N_CORES = 8
B, S, D, H, MLP = 2, 2048, 1024, 16, 4096
DH = D // H                 # 64
T = B * S                   # 4096 tokens
HPC = H // N_CORES          # 2 heads per core
M2 = HPC * DH               # 128 = stacked head dim per core
TSH = T // N_CORES          # 512 tokens per core shard
P = 128
ND = D // P                 # 8 d-chunks
NMC = MLP // P              # 32 mlp chunks
NCH = S // P                # 16 chunks per batch
EPS_LN = 1e-6
KEPS = 1e-3

_CACHE = {}
LAST_RESULT = None


def _layernorm_xhat(nc, pool, eps_sb, x_t, out):
    """out = (x - mean) * rsqrt(var + eps), stats over free dim (D=1024)."""
    stats = pool.tile([P, 2, 6], F32, tag="bnstats")
    xg = x_t.rearrange("p (g d) -> p g d", g=2)
    for g in range(2):
        nc.vector.bn_stats(out=stats[:, g, :], in_=xg[:, g, :])
    mv = pool.tile([P, 2], F32, tag="bnaggr")
    nc.vector.bn_aggr(out=mv, in_=stats)
    nc.scalar.activation(
        out=mv[:, 1:2], in_=mv[:, 1:2],
        func=mybir.ActivationFunctionType.Sqrt,
        bias=eps_sb, scale=1.0)
    nc.vector.reciprocal(out=mv[:, 1:2], in_=mv[:, 1:2])
    nc.vector.tensor_scalar(
        out=out, in0=x_t, scalar1=mv[:, 0:1], scalar2=mv[:, 1:2],
        op0=mybir.AluOpType.subtract, op1=mybir.AluOpType.mult)


def _transpose_1024(nc, tp_ps, ident, src, dst_blk, col0, copy_engine=None):
    """PE-transpose src [128,1024] into dst_blk[:, d, col0:col0+128] (d=0..7)."""
    tp = tp_ps.tile([P, 4, P], BF16)
    for dk in range(ND):
        nc.tensor.transpose(tp[:, dk % 4, :], src[:, dk * P:(dk + 1) * P],
                            ident)
        if dk % 4 == 3:
            dst = dst_blk[:, dk - 3:dk + 1, col0:col0 + P]
            if copy_engine == "scalar":
                nc.scalar.activation(
                    out=dst, in_=tp,
                    func=mybir.ActivationFunctionType.Copy)
            else:
                nc.vector.tensor_copy(out=dst, in_=tp)
            if dk == 3:
                tp = tp_ps.tile([P, 4, P], BF16)


def _build(flags):
    have_ln1b, have_b2 = flags
    nc = bacc.Bacc("TRN2", target_bir_lowering=False, debug=False,
                   num_devices=N_CORES)

    def inp(name, shape, dt=BF16):
        return nc.dram_tensor(name, list(shape), dt, kind="ExternalInput").ap()

    x_dram = inp("x", (T, D))
    xsh_dram = inp("xsh", (TSH, D), F32)     # this core's token shard of inputs
    wq_dram = inp("wq", (P, ND, P))          # [p, d, m] pre-tiled, bf16
    wk_dram = inp("wk", (P, ND, P))
    wv_dram = inp("wv", (P, ND, P))
    wo_dram = inp("wo", (M2, D))
    w1_dram = inp("w1", (NMC, P, ND, P))     # [mc, p, d, j] pre-tiled, bf16
    b1_dram = inp("b1", (NMC, P), F32)       # gelu bias per mlp unit
    w2_dram = inp("w2", (NMC, 2, P, 512))  # [mc, half, p, dout]
    ident_dram = inp("ident", (P, P))
    lmask_dram = inp("lmask", (P, P), F32)   # [j, i] = 1.0 if j <= i
    if have_ln1b:
        qbc_dram = inp("qbc", (M2, 1), F32)
        kbc_dram = inp("kbc", (M2, 1), F32)
        vbc_dram = inp("vbc", (M2, 1), F32)
    if have_b2:
        b2_dram = inp("b2", (1, D), F32)
    out_dram = nc.dram_tensor("out", [TSH, D], F32, kind="ExternalOutput").ap()

    with tile.TileContext(nc) as tc:
        with (
            tc.tile_pool(name="dram", bufs=1, space="DRAM") as dram,
            tc.tile_pool(name="const", bufs=1) as const,
        ):
            partial = dram.tile([T, D], BF16)
            xshard = dram.tile([TSH, D], BF16)

            ident = const.tile([P, P], BF16)
            nc.sync.dma_start(out=ident, in_=ident_dram)
            lmask = const.tile([P, P], F32)
            nc.sync.dma_start(out=lmask, in_=lmask_dram)
            wq_sb = const.tile([P, ND, P], BF16)
            nc.sync.dma_start(out=wq_sb, in_=wq_dram)
            wk_sb = const.tile([P, ND, P], BF16)
            nc.sync.dma_start(out=wk_sb, in_=wk_dram)
            wv_sb = const.tile([P, ND, P], BF16)
            nc.sync.dma_start(out=wv_sb, in_=wv_dram)
            wo_sb = const.tile([M2, D], BF16)
            nc.sync.dma_start(out=wo_sb, in_=wo_dram)
            eps_sb = const.tile([P, 1], F32)
            nc.vector.memset(eps_sb, EPS_LN)
            if have_ln1b:
                qbc_sb = const.tile([M2, 1], F32)
                nc.sync.dma_start(out=qbc_sb, in_=qbc_dram)
                kbc_sb = const.tile([M2, 1], F32)
                nc.sync.dma_start(out=kbc_sb, in_=kbc_dram)
                vbc_sb = const.tile([M2, 1], F32)
                nc.sync.dma_start(out=vbc_sb, in_=vbc_dram)
            if have_b2:
                b2_sb = const.tile([P, D], F32)
                nc.sync.dma_start(out=b2_sb, in_=b2_dram.to_broadcast([P, D]))

            # ---------------- stages 1-3 (head-parallel) ------------------
            with (
                tc.tile_pool(name="xin", bufs=3) as xin_pool,
                tc.tile_pool(name="stats", bufs=4) as stats_pool,
                tc.tile_pool(name="xt", bufs=2) as xt_pool,
                tc.tile_pool(name="tp_ps", bufs=2, space="PSUM") as tp_ps,
                tc.tile_pool(name="mm_ps", bufs=2, space="PSUM") as mm_ps,
                tc.tile_pool(name="qk", bufs=1) as qk_pool,
                tc.tile_pool(name="a_ps", bufs=1, space="PSUM") as a_ps,
                tc.tile_pool(name="ot_ps", bufs=2, space="PSUM") as ot_ps,
                tc.tile_pool(name="st_ps", bufs=1, space="PSUM") as st_ps,
                tc.tile_pool(name="small", bufs=4) as small,
                tc.tile_pool(name="po", bufs=3) as po_pool,
            ):
                qt_phi = qk_pool.tile([P, T], BF16)    # [2 heads x 64, tok]
                kt_phi = qk_pool.tile([P, T], BF16)
                knat = qk_pool.tile([P, T // P, M2], BF16)  # [tok, tile, m]
                vones = qk_pool.tile([P, T // P, HPC, DH + 1], BF16)
                onat = qk_pool.tile([P, T // P, M2], BF16)  # attn [tok, h*dh]
                otn = qk_pool.tile([P, T], BF16)       # normalized attn^T

                for blk in range(T // 512):
                    xt_blk = xt_pool.tile([P, ND, 512], BF16)
                    for lt in range(4):
                        t_idx = blk * 4 + lt
                        r0 = t_idx * P
                        x_t = xin_pool.tile([P, D], BF16)
                        nc.sync.dma_start(out=x_t, in_=x_dram[r0:r0 + P, :])
                        xh = xin_pool.tile([P, D], BF16, tag="xhat")
                        _layernorm_xhat(nc, stats_pool, eps_sb, x_t, xh)
                        _transpose_1024(nc, tp_ps, ident, xh, xt_blk, lt * P,
                                        copy_engine="scalar" if lt % 2 else None)

                    c0 = blk * 512
                    # Qt / Kt / Vt, [m=128, 512] each
                    for w_sb, dst, bc in (
                            (wq_sb, qt_phi, "q"), (wk_sb, kt_phi, "k"),
                            (wv_sb, None, "v")):
                        ps = mm_ps.tile([P, 512], F32, tag="mm")
                        for dk in range(ND):
                            nc.tensor.matmul(ps, w_sb[:, dk, :],
                                             xt_blk[:, dk, :],
                                             start=(dk == 0),
                                             stop=(dk == ND - 1))
                        if bc == "v":
                            vt_sb = xt_pool.tile([P, 512], BF16, tag="vt")
                            if have_ln1b:
                                nc.scalar.activation(
                                    out=vt_sb, in_=ps,
                                    func=mybir.ActivationFunctionType.Copy,
                                    bias=vbc_sb, scale=1.0)
                            else:
                                nc.vector.tensor_copy(out=vt_sb, in_=ps)
                        elif have_ln1b:
                            bias = qbc_sb if bc == "q" else kbc_sb
                            nc.scalar.activation(
                                out=dst[:, c0:c0 + 512], in_=ps,
                                func=mybir.ActivationFunctionType.Relu,
                                bias=bias, scale=1.0)
                            nc.vector.tensor_scalar_add(
                                dst[:, c0:c0 + 512], dst[:, c0:c0 + 512],
                                KEPS)
                        else:
                            nc.vector.tensor_scalar(
                                out=dst[:, c0:c0 + 512], in0=ps,
                                scalar1=0.0, scalar2=KEPS,
                                op0=mybir.AluOpType.max,
                                op1=mybir.AluOpType.add)
                    # K_nat / V_nat via PE transpose of kt_phi / vt
                    for lt in range(4):
                        t_idx = blk * 4 + lt
                        ls = slice(lt * P, (lt + 1) * P)
                        ktp = tp_ps.tile([P, P], BF16, tag="tp")
                        nc.tensor.transpose(
                            ktp, kt_phi[:, c0 + lt * P:c0 + (lt + 1) * P],
                            ident)
                        nc.vector.tensor_copy(out=knat[:, t_idx, :], in_=ktp)
                        vtp = tp_ps.tile([P, P], BF16, tag="tp")
                        nc.tensor.transpose(vtp, vt_sb[:, ls], ident)
                        for h in range(HPC):
                            nc.vector.tensor_copy(
                                out=vones[:, t_idx, h, 0:DH],
                                in_=vtp[:, h * DH:(h + 1) * DH])
                            nc.vector.memset(
                                vones[:, t_idx, h, DH:DH + 1], 1.0)

                # ---------------- attention (chunked causal scan) ---------
                for b in range(B):
                    # both heads' scan state in one 128-partition tile:
                    # rows 64h..64h+63 belong to head h (keeps matmul operand
                    # base partitions aligned with qt_phi/kt_phi head slices)
                    stp = st_ps.tile([P, DH + 1], F32, tag="stp",
                                     name=f"stp{b}")
                    state_sb = None
                    for ch in range(NCH):
                        g0 = b * S + ch * P
                        t_idx = g0 // P
                        cs = slice(g0, g0 + P)
                        prev_state = state_sb
                        if ch < NCH - 1:
                            state_sb = small.tile([P, DH + 1], BF16,
                                                  tag="state",
                                                  name=f"state{b}_{ch}")
                        for h in range(HPC):
                            hr = slice(h * DH, (h + 1) * DH)
                            aps = a_ps.tile([P, P], F32)
                            nc.tensor.matmul(aps, kt_phi[hr, cs],
                                             qt_phi[hr, cs],
                                             start=True, stop=True)
                            am = small.tile([P, P], BF16, tag="am")
                            nc.vector.tensor_tensor(
                                out=am, in0=aps, in1=lmask,
                                op=mybir.AluOpType.mult)
                            ops = ot_ps.tile([P, DH + 1], F32)
                            nc.tensor.matmul(ops, am, vones[:, t_idx, h, :],
                                             start=True, stop=(ch == 0))
                            if ch > 0:
                                nc.tensor.matmul(ops, qt_phi[hr, cs],
                                                 prev_state[hr, :],
                                                 start=False, stop=True,
                                                 skip_group_check=True)
                            if ch < NCH - 1:
                                nc.tensor.matmul(
                                    stp[hr, :], knat[:, t_idx, hr],
                                    vones[:, t_idx, h, :],
                                    start=(ch == 0), stop=(ch == NCH - 2),
                                    skip_group_check=True)
                                nc.vector.tensor_copy(out=state_sb[hr, :],
                                                      in_=stp[hr, :])
                            rec = small.tile([P, 1], F32, tag="rec")
                            nc.vector.reciprocal(out=rec,
                                                 in_=ops[:, DH:DH + 1])
                            nc.vector.tensor_scalar_mul(
                                onat[:, t_idx, hr], ops[:, 0:DH], rec)
                        # transpose both heads [tok,128] -> otn [128, tok]
                        otp = tp_ps.tile([P, P], BF16, tag="tp")
                        nc.tensor.transpose(otp, onat[:, t_idx, :], ident)
                        nc.vector.tensor_copy(out=otn[:, cs], in_=otp)

                # ---------------- output projection partials --------------
                for t_idx in range(T // P):
                    r0 = t_idx * P
                    posb = po_pool.tile([P, D], BF16)
                    for half in range(2):
                        pops = mm_ps.tile([P, 512], F32, tag="mm")
                        nc.tensor.matmul(
                            pops, otn[:, r0:r0 + P],
                            wo_sb[:, half * 512:(half + 1) * 512],
                            start=True, stop=True)
                        nc.scalar.activation(
                            out=posb[:, half * 512:(half + 1) * 512],
                            in_=pops,
                            func=mybir.ActivationFunctionType.Copy)
                    nc.sync.dma_start(out=partial[r0:r0 + P, :], in_=posb)

            nc.gpsimd.collective_compute(
                "ReduceScatter",
                mybir.AluOpType.add,
                replica_groups=[list(range(N_CORES))],
                ins=[partial.opt()],
                outs=[xshard.opt()],
            )

            # ---------------- stage 4: token-local MLP --------------------
            with (
                tc.tile_pool(name="xs", bufs=4) as xs_pool,
                tc.tile_pool(name="ys", bufs=2) as ys_pool,
                tc.tile_pool(name="stats2", bufs=4) as stats2_pool,
                tc.tile_pool(name="yt", bufs=1) as yt_pool,
                tc.tile_pool(name="tp2_ps", bufs=2, space="PSUM") as tp2_ps,
                tc.tile_pool(name="h_sb", bufs=1) as h_pool,
                tc.tile_pool(name="w1t", bufs=6) as w1t_pool,
                tc.tile_pool(name="w2t", bufs=4) as w2t_pool,
                tc.tile_pool(name="b1t", bufs=3) as b1t_pool,
                tc.tile_pool(name="h_ps", bufs=2, space="PSUM") as h_ps,
                tc.tile_pool(name="y2_ps", bufs=4, space="PSUM") as y2_ps,
                tc.tile_pool(name="fin", bufs=4) as fin_pool,
            ):
                yt_blk = yt_pool.tile([P, ND, TSH], BF16)
                x_tiles = []
                for lt in range(TSH // P):
                    r0 = lt * P
                    xsb = xs_pool.tile([P, D], BF16, tag="xsb")
                    nc.sync.dma_start(out=xsb, in_=xshard[r0:r0 + P, :])
                    xs = xs_pool.tile([P, D], F32)
                    nc.sync.dma_start(out=xs, in_=xsh_dram[r0:r0 + P, :])
                    nc.vector.tensor_add(xs, xs, xsb)
                    x_tiles.append(xs)
                    ysb = ys_pool.tile([P, D], BF16)
                    _layernorm_xhat(nc, stats2_pool, eps_sb, xs, ysb)
                    _transpose_1024(nc, tp2_ps, ident, ysb, yt_blk, r0,
                                    copy_engine="scalar" if lt % 2 else None)

                h_all = h_pool.tile([P, NMC, TSH], BF16)
                for mc in range(NMC):
                    w1t = w1t_pool.tile([P, ND, P], BF16)
                    nc.sync.dma_start(out=w1t, in_=w1_dram[mc])
                    b1t = b1t_pool.tile([P, 1], F32)
                    nc.sync.dma_start(
                        out=b1t, in_=b1_dram[mc].rearrange("(p o) -> p o", o=1))
                    hps = h_ps.tile([P, TSH], F32)
                    for dk in range(ND):
                        nc.tensor.matmul(hps, w1t[:, dk, :], yt_blk[:, dk, :],
                                         start=(dk == 0), stop=(dk == ND - 1))
                    nc.scalar.activation(
                        out=h_all[:, mc, :], in_=hps,
                        func=mybir.ActivationFunctionType.Gelu_apprx_tanh,
                        bias=b1t, scale=1.0)

                fins = [fin_pool.tile([P, D], F32, tag="fin", name=f"fin{_l}")
                        for _l in range(TSH // P)]
                for half in range(2):
                    hs0 = half * 512
                    y2p = [y2_ps.tile([P, 512], F32, tag="y2",
                                      name=f"y2_{half}_{_l}")
                           for _l in range(TSH // P)]
                    for mc in range(NMC):
                        w2t = w2t_pool.tile([P, 512], BF16)
                        nc.sync.dma_start(out=w2t, in_=w2_dram[mc, half])
                        for lt in range(TSH // P):
                            r0 = lt * P
                            nc.tensor.matmul(
                                y2p[lt], h_all[:, mc, r0:r0 + P], w2t,
                                start=(mc == 0), stop=(mc == NMC - 1))
                    for lt in range(TSH // P):
                        nc.vector.tensor_tensor(
                            out=fins[lt][:, hs0:hs0 + 512], in0=y2p[lt],
                            in1=x_tiles[lt][:, hs0:hs0 + 512],
                            op=mybir.AluOpType.add)
                        if have_b2:
                            nc.vector.tensor_add(
                                fins[lt][:, hs0:hs0 + 512],
                                fins[lt][:, hs0:hs0 + 512],
                                b2_sb[:, hs0:hs0 + 512])
                for lt in range(TSH // P):
                    nc.sync.dma_start(out=out_dram[lt * P:(lt + 1) * P, :],
                                      in_=fins[lt])

    nc.compile()
    return nc


def kernel(**inputs):
    inp = np.asarray(inputs["inputs"], np.float32).reshape(T, D)
    ln1s = np.asarray(inputs["ln1_scale"], np.float32)
    ln1b = np.asarray(inputs["ln1_bias"], np.float32)
    wq = np.asarray(inputs["wq"], np.float32).reshape(D, H * DH)
    wk = np.asarray(inputs["wk"], np.float32).reshape(D, H * DH)
    wv = np.asarray(inputs["wv"], np.float32).reshape(D, H * DH)
    wo = np.asarray(inputs["wo"], np.float32).reshape(H * DH, D)
    ln2s = np.asarray(inputs["ln2_scale"], np.float32)
    ln2b = np.asarray(inputs["ln2_bias"], np.float32)
    w1 = np.asarray(inputs["w1"], np.float32)
    b1 = np.asarray(inputs["b1"], np.float32)
    w2 = np.asarray(inputs["w2"], np.float32)
    b2 = np.asarray(inputs["b2"], np.float32)

    wq_e = ln1s[:, None] * wq
    wk_e = ln1s[:, None] * wk
    wv_e = ln1s[:, None] * wv
    qb = ln1b @ wq
    kb = ln1b @ wk
    vb = ln1b @ wv
    w1_e = ln2s[:, None] * w1
    b1_e = ln2b @ w1 + b1

    have_ln1b = bool(np.any(qb) or np.any(kb) or np.any(vb))
    have_b2 = bool(np.any(b2))
    key = (have_ln1b, have_b2)
    if key not in _CACHE:
        _CACHE[key] = _build(key)
    nc = _CACHE[key]

    def bf(a):
        return np.ascontiguousarray(a.astype(NP_BF16))

    def tile_w(w):  # [D, 128] -> [p, d, j] pre-tiled
        return bf(w.reshape(ND, P, P).transpose(1, 0, 2))

    w1_tiled = bf(w1_e.reshape(ND, P, NMC, P).transpose(2, 1, 0, 3))
    w2_bf = bf(w2.reshape(NMC, P, 2, 512).transpose(0, 2, 1, 3))
    in_maps = []
    for c in range(N_CORES):
        hs = slice(c * M2, (c + 1) * M2)
        m = {
            "x": bf(inp),
            "xsh": np.ascontiguousarray(inp[c * TSH:(c + 1) * TSH]),
            "wq": tile_w(wq_e[:, hs]),
            "wk": tile_w(wk_e[:, hs]),
            "wv": tile_w(wv_e[:, hs]),
            "wo": bf(wo[hs, :]),
            "w1": w1_tiled,
            "b1": np.ascontiguousarray(b1_e.reshape(NMC, P)),
            "w2": w2_bf,
            "ident": np.eye(P, dtype=NP_BF16),
            "lmask": np.ascontiguousarray(
                np.tril(np.ones((P, P), np.float32)).T),
        }
        if have_ln1b:
            m["qbc"] = np.ascontiguousarray(qb[hs].reshape(M2, 1))
            m["kbc"] = np.ascontiguousarray(kb[hs].reshape(M2, 1))
            m["vbc"] = np.ascontiguousarray(vb[hs].reshape(M2, 1))
        if have_b2:
            m["b2"] = b2.reshape(1, D)
        in_maps.append(m)

    global LAST_RESULT
    res = run_bass_kernel_spmd(nc, in_maps, core_ids=list(range(N_CORES)))
    LAST_RESULT = res
    out = np.concatenate([res.results[c]["out"] for c in range(N_CORES)], 0)
    return out.reshape(B, S, D).astype(np.float32)
